# revision 1
# baseline (speedup 1.0000x reference)
"""Trainium2 Bass kernel for nn_CompresSAEEncoder.

Reference computation (per row i of x):
    xn = x / ||x||_2
    e  = xn @ W + b                      # [B, D_EMB]
    keep top-k of |e| per row, zero the rest (signs preserved)

Sharding: data-parallel over the batch dim across 8 NeuronCores.
Each core gets 1024 rows of x (plus pre-transposed/pre-rounded copies
for the matmul operands) and a full copy of W/b, computes its
1024x32768 output slice fully locally (top-k is per-row, so row
sharding needs no collectives), and the host concatenates the slices.

Matmul precision: the PE's fast fp32 path (float32r) rounds operands
to a 12-bit mantissa, which perturbs |e| enough to flip ~1.4% of rows'
top-64 boundary vs the fp32 reference.  PRECISE mode therefore uses a
3-term compensated product at full-rate (1 cycle/row) dtypes:

    e ~= fx@fW + dx@bW + bx@dW        (error ~2^-22, fp32-grade)

with fx=fp32r(x), dx=bf16(x-fx), bx=bf16(x), and same for W.  The two
residual terms are bf16 matmuls; all three accumulate into the same
PSUM tile.  3x the MACs of one pass, still ~78 TF/s/core dense.

Per-core pipeline (per 512-row pass; 2 passes per core in PRECISE
mode, 1x1024-row pass in fast mode):
  A) row norms: ACT Square+accum over x rows -> sqrt -> reciprocal
  B) for each 512-wide column block: 32 (x3) matmuls accumulate in
     PSUM; DVE evicts with fused (psum * rn + b) -> e strip; e strip
     spills to DRAM; ACT computes |e|; DVE max8/match_replace keeps the
     top-16 |e| of each strip as top-k candidates.
     (exactness: the row's top-64 lie in the union of per-strip top-16
     unless one 512-strip holds >16 of the row's top-64 -- for
     continuous random data that probability is ~1e-15 per row)
  C) per row-tile: 8x(max8+match_replace) over the 1024 candidates
     gives tau = 64th largest |e|; re-read e, out = (|e| >= tau) * e.
"""

from contextlib import ExitStack

import numpy as np

import concourse.bacc as bacc
import concourse.mybir as mybir
from concourse.bass_utils import run_bass_kernel_spmd
from concourse.tile import TileContext

F32 = mybir.dt.float32
F32R = mybir.dt.float32r
BF16 = mybir.dt.bfloat16

# Full problem shape (hardcoded per contest contract).
B, D_IN, D_EMB, K = 8192, 4096, 32768, 64
N_CORES = 8
ROWS = B // N_CORES

PRECISE = True  # 3-term compensated matmul (fp32-grade selection accuracy)


def _emit_pass(nc, tc, tensors, r0, rows, d_in, d_emb, k, keep, strip, precise):
    """Emit phases A-C for rows [r0, r0+rows) of this core's shard."""
    RT = rows // 128
    KT = d_in // 128
    CB = d_emb // strip
    AF = mybir.ActivationFunctionType
    OP = mybir.AluOpType
    (xt_d, dxt_d, xbt_d, x_d, w_d, wb_d, dw_d, b_d, out_d, esp_d) = tensors

    with ExitStack() as stack:
        perm = stack.enter_context(
            tc.tile_pool(name=f"perm{r0}", bufs=1))
        cand = perm.tile([128, RT * CB * keep], F32, tag="cand",
                         name=f"cand{r0}")
        ss = perm.tile([128, RT], F32, tag="ss", name=f"ss{r0}")
        rn = perm.tile([128, RT], F32, tag="rn", name=f"rn{r0}")
        srt = perm.tile([128, RT], F32, tag="srt", name=f"srt{r0}")

        xt_stack = ExitStack()
        xt_pool = xt_stack.enter_context(
            tc.tile_pool(name=f"xt_pool{r0}", bufs=1))
        # resident stationary operands: k-tile kk at cols [kk*rows, ...)
        xt_sb = xt_pool.tile([128, KT * rows], F32R, tag="xt_res",
                             name=f"xt_sb{r0}")
        for kk in range(KT):
            nc.sync.dma_start(
                out=xt_sb[:, kk * rows:(kk + 1) * rows],
                in_=xt_d[kk * 128:(kk + 1) * 128, r0:r0 + rows])
        if precise:
            dxt_sb = xt_pool.tile([128, KT * rows], BF16, tag="dxt_res",
                                  name=f"dxt_sb{r0}")
            xbt_sb = xt_pool.tile([128, KT * rows], BF16, tag="xbt_res",
                                  name=f"xbt_sb{r0}")
            for kk in range(KT):
                nc.sync.dma_start(
                    out=dxt_sb[:, kk * rows:(kk + 1) * rows],
                    in_=dxt_d[kk * 128:(kk + 1) * 128, r0:r0 + rows])
                nc.sync.dma_start(
                    out=xbt_sb[:, kk * rows:(kk + 1) * rows],
                    in_=xbt_d[kk * 128:(kk + 1) * 128, r0:r0 + rows])

        # --- phase A: row norms ------------------------------------------
        with tc.tile_pool(name=f"norm{r0}", bufs=1) as npool:
            for t in range(RT):
                xtile = npool.tile([128, d_in], F32, tag="xtile", bufs=2,
                                   name=f"xtile{r0}_{t}")
                nc.sync.dma_start(
                    out=xtile,
                    in_=x_d[r0 + t * 128:r0 + (t + 1) * 128, :])
                scr = npool.tile([128, d_in], BF16, tag="sq_scr",
                                 name=f"scr{r0}_{t}")
                nc.scalar.activation(out=scr, in_=xtile, func=AF.Square,
                                     accum_out=ss[:, t:t + 1])
            nc.scalar.activation(out=srt, in_=ss, func=AF.Sqrt)
            nc.vector.reciprocal(out=rn, in_=srt)

        # --- phase B: matmuls + evict + spill + strip candidates ---------
        with tc.tile_pool(name=f"wbuf{r0}", bufs=6) as wpool, \
             tc.tile_pool(name=f"bbuf{r0}", bufs=2) as bpool, \
             tc.tile_pool(name=f"zbuf{r0}", bufs=6) as zpool, \
             tc.tile_pool(name=f"abuf{r0}", bufs=4) as apool, \
             tc.tile_pool(name=f"psum{r0}", bufs=8, space="PSUM") as ppool:
            for cb in range(CB):
                c0 = cb * strip
                psums = [
                    ppool.tile([128, strip], F32, tag="ps",
                               name=f"ps{r0}_{cb}_{t}")
                    for t in range(RT)
                ]
                streams = [(xt_sb, w_d, F32R)]
                if precise:
                    streams += [(dxt_sb, wb_d, BF16), (xbt_sb, dw_d, BF16)]
                n_streams = len(streams)
                for si, (lhs_sb, rhs_d, dt) in enumerate(streams):
                    for kk in range(KT):
                        wt = wpool.tile([128, strip], dt, tag=f"wt{si}",
                                        name=f"wt{r0}_{si}_{cb}_{kk}")
                        nc.sync.dma_start(
                            out=wt,
                            in_=rhs_d[kk * 128:(kk + 1) * 128, c0:c0 + strip])
                        for t in range(RT):
                            lhs = lhs_sb[:, kk * rows + t * 128:
                                         kk * rows + (t + 1) * 128]
                            nc.tensor.matmul(
                                psums[t], lhs, wt,
                                start=(si == 0 and kk == 0),
                                stop=(si == n_streams - 1 and kk == KT - 1))
                bb = bpool.tile([128, strip], F32, tag="bb",
                                name=f"bb{r0}_{cb}")
                nc.sync.dma_start(
                    out=bb, in_=b_d[0:128, c0:c0 + strip])
                for t in range(RT):
                    zb = zpool.tile([128, strip], F32, tag="zb",
                                    name=f"zb{r0}_{cb}_{t}")
                    # e = psum * (1/norm) + b
                    nc.vector.scalar_tensor_tensor(
                        out=zb, in0=psums[t], scalar=rn[:, t:t + 1], in1=bb,
                        op0=OP.mult, op1=OP.add)
                    nc.sync.dma_start(
                        out=esp_d[r0 + t * 128:r0 + (t + 1) * 128,
                                  c0:c0 + strip],
                        in_=zb)
                    ab = apool.tile([128, strip], F32, tag="ab",
                                    name=f"ab{r0}_{cb}_{t}")
                    nc.scalar.activation(out=ab, in_=zb, func=AF.Abs)
                    slot = t * CB * keep + cb * keep
                    nc.vector.max(out=cand[:, slot:slot + 8], in_=ab)
                    if keep > 8:
                        ab2 = apool.tile([128, strip], F32, tag="ab2",
                                         name=f"ab2{r0}_{cb}_{t}")
                        nc.vector.match_replace(
                            out=ab2, in_to_replace=cand[:, slot:slot + 8],
                            in_values=ab, imm_value=-1.0)
                        nc.vector.max(out=cand[:, slot + 8:slot + 16],
                                      in_=ab2)

        xt_stack.close()  # release resident operands before phase C

        # --- phase C: merge candidates -> tau; mask ----------------------
        QW = min(4096, d_emb)
        NQ = d_emb // QW
        with tc.tile_pool(name=f"vpool{r0}", bufs=2) as vpool, \
             tc.tile_pool(name=f"mwork{r0}", bufs=2) as mpool, \
             tc.tile_pool(name=f"equart{r0}", bufs=3) as epool, \
             tc.tile_pool(name=f"aquart{r0}", bufs=2) as aqpool, \
             tc.tile_pool(name=f"oquart{r0}", bufs=2) as opool:
            for t in range(RT):
                nc8 = CB * keep
                creg = cand[:, t * nc8:(t + 1) * nc8]
                vv = vpool.tile([128, ((k + 7) // 8) * 8], F32, tag="vv",
                                name=f"vv{r0}_{t}")
                work_a = mpool.tile([128, nc8], F32, tag="mwa",
                                    name=f"mwa{r0}_{t}")
                work_b = mpool.tile([128, nc8], F32, tag="mwb",
                                    name=f"mwb{r0}_{t}")
                src = creg
                rounds = (k + 7) // 8
                for r in range(rounds):
                    nc.vector.max(out=vv[:, r * 8:(r + 1) * 8], in_=src)
                    if r < rounds - 1:
                        dst = work_a if r % 2 == 0 else work_b
                        nc.vector.match_replace(
                            out=dst, in_to_replace=vv[:, r * 8:(r + 1) * 8],
                            in_values=src, imm_value=-1.0)
                        src = dst
                tau = vv[:, k - 1:k]
                for q in range(NQ):
                    q0 = q * QW
                    eq = epool.tile([128, QW], F32, tag="eq",
                                    name=f"eq{r0}_{t}_{q}")
                    nc.sync.dma_start(
                        out=eq,
                        in_=esp_d[r0 + t * 128:r0 + (t + 1) * 128,
                                  q0:q0 + QW])
                    aq = aqpool.tile([128, QW], F32, tag="aq",
                                     name=f"aq{r0}_{t}_{q}")
                    nc.scalar.activation(out=aq, in_=eq, func=AF.Abs)
                    oq = opool.tile([128, QW], F32, tag="oq",
                                    name=f"oq{r0}_{t}_{q}")
                    # out = (|e| >= tau) * e
                    nc.vector.scalar_tensor_tensor(
                        out=oq, in0=aq, scalar=tau, in1=eq,
                        op0=OP.is_ge, op1=OP.mult)
                    nc.sync.dma_start(
                        out=out_d[r0 + t * 128:r0 + (t + 1) * 128,
                                  q0:q0 + QW],
                        in_=oq)


def build_nc(rows, d_in, d_emb, k, keep=16, strip=512, precise=PRECISE):
    """Build the single-core Bass program (all 8 cores run it SPMD)."""
    assert rows % 128 == 0 and d_in % 128 == 0 and d_emb % strip == 0
    assert (d_emb // strip) * keep >= k

    nc = bacc.Bacc("TRN2", target_bir_lowering=False)

    # xt/w arrive pre-rounded to fp32r (12-bit mantissa) from the host;
    # declaring them float32r end-to-end satisfies the BIR verifier's
    # "matmul input must be rounded to FP32r" check.
    xt_d = nc.dram_tensor("xt", [d_in, rows], F32R, kind="ExternalInput")
    x_d = nc.dram_tensor("xn", [rows, d_in], F32, kind="ExternalInput")
    w_d = nc.dram_tensor("w", [d_in, d_emb], F32R, kind="ExternalInput")
    # b pre-replicated to 128 partitions on the host (tiny)
    b_d = nc.dram_tensor("b", [128, d_emb], F32, kind="ExternalInput")
    dxt_d = xbt_d = wb_d = dw_d = None
    if precise:
        dxt_d = nc.dram_tensor("dxt", [d_in, rows], BF16, kind="ExternalInput")
        xbt_d = nc.dram_tensor("xbt", [d_in, rows], BF16, kind="ExternalInput")
        wb_d = nc.dram_tensor("wb", [d_in, d_emb], BF16, kind="ExternalInput")
        dw_d = nc.dram_tensor("dw", [d_in, d_emb], BF16, kind="ExternalInput")
    out_d = nc.dram_tensor("out", [rows, d_emb], F32, kind="ExternalOutput")
    esp_d = nc.dram_tensor("espill", [rows, d_emb], F32)  # Internal scratch

    tensors = (xt_d, dxt_d, xbt_d, x_d, w_d, wb_d, dw_d, b_d, out_d, esp_d)
    # PRECISE needs 3 resident stationary tensors; halve the row block so
    # they fit in SBUF.
    pass_rows = min(rows, 512 if precise else 1024)
    with TileContext(nc) as tc:
        for r0 in range(0, rows, pass_rows):
            _emit_pass(nc, tc, tensors, r0, pass_rows, d_in, d_emb, k,
                       keep, strip, precise)
    nc.compile()  # bacc legalization: wait-splitting via event sems, etc.
    return nc


def round_fp32r(a):
    """Round fp32 -> fp32r (round-to-nearest to 11-bit mantissa), matching
    walrus's fp32_to_fp32r."""
    u = np.ascontiguousarray(a).view(np.uint32).astype(np.uint64)
    r = ((u + 0x800) & 0xFFFFF000) & 0xFFFFFFFF
    return r.astype(np.uint32).view(np.float32)


def to_bf16(a):
    import ml_dtypes
    return a.astype(ml_dtypes.bfloat16)


_NC_CACHE = {}


def _get_nc(rows, d_in, d_emb, k, precise):
    key = (rows, d_in, d_emb, k, precise)
    if key not in _NC_CACHE:
        _NC_CACHE[key] = build_nc(rows, d_in, d_emb, k, precise=precise)
    return _NC_CACHE[key]


def kernel(x, W, b, k, _trace=False, _precise=PRECISE):
    """Full-input entry point: shards across 8 NeuronCores internally."""
    x = np.asarray(x, dtype=np.float32)
    W = np.ascontiguousarray(np.asarray(W, dtype=np.float32))
    b = np.ascontiguousarray(np.asarray(b, dtype=np.float32)).reshape(1, -1)
    kk = int(np.asarray(k))
    Bfull, d_in = x.shape
    d_emb = W.shape[1]
    assert (Bfull, d_in, d_emb, kk) == (B, D_IN, D_EMB, K), (
        f"kernel hardcoded for {(B, D_IN, D_EMB, K)}, got "
        f"{(Bfull, d_in, d_emb, kk)}")

    rows = Bfull // N_CORES
    nc = _get_nc(rows, d_in, d_emb, kk, _precise)

    Wr = round_fp32r(W)
    b_rep = np.ascontiguousarray(np.broadcast_to(b, (128, d_emb)))
    base = {"w": Wr, "b": b_rep}
    if _precise:
        base["wb"] = to_bf16(W)
        base["dw"] = to_bf16(W - Wr)
    in_maps = []
    for c in range(N_CORES):
        xc = np.ascontiguousarray(x[c * rows:(c + 1) * rows])
        xct = np.ascontiguousarray(xc.T)
        xr = round_fp32r(xct)
        m = {"xt": xr, "xn": xc, **base}
        if _precise:
            m["dxt"] = to_bf16(xct - xr)
            m["xbt"] = to_bf16(xct)
        in_maps.append(m)
    res = run_bass_kernel_spmd(
        nc, in_maps, core_ids=list(range(N_CORES)), trace=_trace)
    out = np.concatenate([res.results[c]["out"] for c in range(N_CORES)],
                         axis=0)
    if _trace:
        return out, res
    return out



# revision 7
# speedup vs baseline: 1.6579x; 1.6579x over previous
"""Trainium2 Bass kernel for nn_CompresSAEEncoder.

Reference computation (per row i of x):
    xn = x / ||x||_2
    e  = xn @ W + b                      # [B, D_EMB]
    keep top-k of |e| per row, zero the rest (signs preserved)

Sharding: data-parallel over the batch dim across 8 NeuronCores.
Each core gets 1024 rows of x and a full copy of W, computes its
1024x32768 output slice fully locally (top-k is per-row, so row
sharding needs no collectives), and the host concatenates the slices.

Matmul precision: single fp32r (11-bit mantissa) main pass at full PE
rate, plus ONE fp8e4(DoubleRow, 2x rate) correction matmul that fixes
BOTH operands' fp32r rounding errors simultaneously:

    u  = f8(xr + s*(x - xr))       (s = 2^11)
    V  = f8(c*(Wr + s*(W - Wr)))   (c = 64 keeps fp8 in range)
    e ~= (1 - 1/s) * (xr @ Wr) + (u @ V) / (c*(s-1))

Residual error ~4.5e-7 vs e std 1.56e-2 (2.9e-5 relative): ~6 top-64
boundary flips per 8192 rows -> rel_err ~9e-3, vs 0.0193 for fp32r
alone (116 flips) against the 2e-2 gate.

Per-core pipeline (single pass over 64 column blocks of 512):
  A) row norms from x: ACT Square+accum -> sqrt -> reciprocal
     (the (1-1/s) and 1/(c*(s-1)) factors fold into the norm scalars)
  B) per column block: 16 fp8-DoubleRow corr matmuls accumulate in
     PSUM; ACT evicts ct = corr * rnc (bf16); 32 fp32r main matmuls
     reuse the same PSUM banks; DVE evicts e = main*rn' + ct; spill e
     to DRAM; ACT abs; DVE max8 keeps the top-8 |e| of each strip.
     (union of per-strip top-8 covers the row top-64 up to ~1e-6/row)
  C) per row-tile: 8x(max8+match_replace) over the 512 candidates
     gives tau = 64th largest |e|; re-read e, out = (|e| >= tau) * e.
"""

from contextlib import ExitStack

import numpy as np

import concourse.bacc as bacc
import concourse.mybir as mybir
from concourse.bass_utils import run_bass_kernel_spmd
from concourse.tile import TileContext

F32 = mybir.dt.float32
F32R = mybir.dt.float32r
BF16 = mybir.dt.bfloat16
FP8 = mybir.dt.float8e4

# Full problem shape (hardcoded per contest contract).
B, D_IN, D_EMB, K = 8192, 4096, 32768, 64
N_CORES = 8
ROWS = B // N_CORES

CS = 2048.0  # correction split scale s = 2^11
CC = 64.0    # fp8 range scale for the W-side correction operand

KEEP = 8
STRIP = 512
NKK = 2   # fp32r W k-tiles (128 rows) per DMA chunk
NK2 = 2   # fp8 k2-tiles (256 rows) per DMA chunk
QW = 1024  # phase-C column tile width


def _emit(nc, tc, tensors, rows, d_in, d_emb, k, has_bias):
    RT = rows // 128
    KT = d_in // 128
    K2T = KT // 2
    CB = d_emb // STRIP
    AF = mybir.ActivationFunctionType
    OP = mybir.AluOpType
    DR = mybir.MatmulPerfMode.DoubleRow
    (xt_d, u8_d, x_d, w_d, v8_d, b_d, out_d, esp_d) = tensors

    with ExitStack() as stack:
        perm = stack.enter_context(tc.tile_pool(name="perm", bufs=1))
        ss = perm.tile([128, RT], F32, tag="ss", name="ss")
        srt = perm.tile([128, RT], F32, tag="srt", name="srt")
        rn1 = perm.tile([128, RT], F32, tag="rn1", name="rn1")  # rn*(1-1/s)
        rnc = perm.tile([128, RT], F32, tag="rnc", name="rnc")  # rn1/(c*(s-1))

        # resident stationary operands, kk-major: k-tile kk at cols
        # [kk*rows, (kk+1)*rows); cand lives from phase B through C
        xt_sb = perm.tile([128, KT * rows], F32R, tag="xt_res", name="xt_sb")
        u8_sb = perm.tile([128, KT * rows], FP8, tag="u8_res", name="u8_sb")
        cand = perm.tile([128, RT * CB * KEEP], F32, tag="cand", name="cand")
        nc.sync.dma_start(
            out=xt_sb.rearrange("p (kk r) -> p kk r", kk=KT),
            in_=xt_d[:, :].rearrange("(kk p) r -> p kk r", p=128))
        nc.sync.dma_start(
            out=u8_sb.rearrange("p (kk r) -> p kk r", kk=KT),
            in_=u8_d[:, :].rearrange("(kk p) r -> p kk r", p=128))
        xt3 = xt_sb.rearrange("p (kk r) -> p kk r", kk=KT)
        u83 = u8_sb.rearrange("p (kk r) -> p kk r", kk=KT)

        # --- phase A: row norm scalars ------------------------------------
        with tc.tile_pool(name="norm", bufs=1) as npool:
            for t in range(RT):
                xtile = npool.tile([128, d_in], F32, tag="xtile",
                                   name=f"xtile{t}")
                nc.sync.dma_start(
                    out=xtile, in_=x_d[t * 128:(t + 1) * 128, :])
                scr = npool.tile([128, d_in], BF16, tag="sq_scr",
                                 name=f"scr{t}")
                nc.scalar.activation(out=scr, in_=xtile, func=AF.Square,
                                     accum_out=ss[:, t:t + 1])
            # srt = ||x|| / (1-1/s);  rn1 = (1-1/s)/||x||
            fac = 1.0 - 1.0 / CS
            nc.scalar.activation(out=srt, in_=ss, func=AF.Sqrt,
                                 scale=1.0 / (fac * fac))
            nc.vector.reciprocal(out=rn1, in_=srt)
            nc.vector.scalar_tensor_tensor(
                out=rnc, in0=rn1, scalar=1.0 / (CC * (CS - 1.0)), in1=rn1,
                op0=OP.mult, op1=OP.bypass)

        # --- phase B: matmuls + evict + spill + strip candidates ----------
        with tc.tile_pool(name="wbuf", bufs=2) as wpool, \
             tc.tile_pool(name="w8buf", bufs=2) as w8pool, \
             tc.tile_pool(name="ctbuf", bufs=1) as ctpool, \
             tc.tile_pool(name="zbuf", bufs=3) as zpool, \
             tc.tile_pool(name="abuf", bufs=2) as apool, \
             tc.tile_pool(name="bbuf", bufs=2) as bpool, \
             tc.tile_pool(name="psum", bufs=8, space="PSUM") as ppool:
            for cb in range(CB):
                c0 = cb * STRIP
                # correction sweep: fp8 DoubleRow, K=256 per instruction
                cps = [ppool.tile([128, STRIP], F32, tag="ps",
                                  name=f"cp{cb}_{t}") for t in range(RT)]
                for ch in range(K2T // NK2):
                    w8t = w8pool.tile([128, NK2 * 2 * STRIP], FP8, tag="w8",
                                      name=f"w8_{cb}_{ch}")
                    nc.scalar.dma_start(
                        out=w8t.rearrange("p (kk c) -> p kk c", kk=2 * NK2),
                        in_=v8_d[ch * NK2 * 256:(ch + 1) * NK2 * 256,
                                 c0:c0 + STRIP]
                        .rearrange("(kk p) c -> p kk c", p=128))
                    for j in range(NK2):
                        k2 = ch * NK2 + j
                        rhs = w8t.rearrange("p (kk c) -> p kk c",
                                            kk=2 * NK2)[:, 2 * j:2 * j + 2, :]
                        for t in range(RT):
                            lhsT = u83[:, 2 * k2:2 * k2 + 2,
                                       t * 128:(t + 1) * 128]
                            nc.tensor.matmul(
                                cps[t], lhsT, rhs, perf_mode=DR,
                                start=(k2 == 0), stop=(k2 == K2T - 1))
                # evict correction (scaled by rn1/(c*(s-1))) as bf16
                cts = []
                for t in range(RT):
                    ct = ctpool.tile([128, STRIP], BF16, tag=f"ct{t}",
                                     name=f"ct{cb}_{t}")
                    nc.scalar.activation(out=ct, in_=cps[t], func=AF.Copy,
                                         scale=rnc[:, t:t + 1])
                    cts.append(ct)
                # main sweep: fp32r full rate
                mps = [ppool.tile([128, STRIP], F32, tag="ps",
                                  name=f"mp{cb}_{t}") for t in range(RT)]
                for ch in range(KT // NKK):
                    wt = wpool.tile([128, NKK * STRIP], F32R, tag="wt",
                                    name=f"wt{cb}_{ch}")
                    nc.sync.dma_start(
                        out=wt.rearrange("p (kk c) -> p kk c", kk=NKK),
                        in_=w_d[ch * NKK * 128:(ch + 1) * NKK * 128,
                                c0:c0 + STRIP]
                        .rearrange("(kk p) c -> p kk c", p=128))
                    for j in range(NKK):
                        kk = ch * NKK + j
                        rhs = wt[:, j * STRIP:(j + 1) * STRIP]
                        for t in range(RT):
                            lhsT = xt3[:, kk, t * 128:(t + 1) * 128]
                            nc.tensor.matmul(
                                mps[t], lhsT, rhs,
                                start=(kk == 0), stop=(kk == KT - 1))
                if has_bias:
                    bb = bpool.tile([128, STRIP], F32, tag="bb",
                                    name=f"bb{cb}")
                    nc.sync.dma_start(out=bb, in_=b_d[0:128, c0:c0 + STRIP])
                for t in range(RT):
                    zb = zpool.tile([128, STRIP], F32, tag="zb",
                                    name=f"zb{cb}_{t}")
                    # e = main * rn1 + ct   (ct already carries rn1)
                    nc.vector.scalar_tensor_tensor(
                        out=zb, in0=mps[t], scalar=rn1[:, t:t + 1],
                        in1=cts[t], op0=OP.mult, op1=OP.add)
                    if has_bias:
                        zb2 = zpool.tile([128, STRIP], F32, tag="zb",
                                         name=f"zbb{cb}_{t}")
                        nc.vector.scalar_tensor_tensor(
                            out=zb2, in0=zb, scalar=1.0, in1=bb,
                            op0=OP.mult, op1=OP.add)
                        zb = zb2
                    nc.scalar.dma_start(
                        out=esp_d[t * 128:(t + 1) * 128, c0:c0 + STRIP],
                        in_=zb)
                    ab = apool.tile([128, STRIP], F32, tag="ab",
                                    name=f"ab{cb}_{t}")
                    nc.scalar.activation(out=ab, in_=zb, func=AF.Abs)
                    slot = t * CB * KEEP + cb * KEEP
                    nc.vector.max(out=cand[:, slot:slot + KEEP], in_=ab)

        # --- phase C: merge candidates -> tau; mask -----------------------
        NQ = d_emb // QW
        with tc.tile_pool(name="vpool", bufs=1) as vpool, \
             tc.tile_pool(name="mwork", bufs=1) as mpool, \
             tc.tile_pool(name="equart", bufs=2) as epool, \
             tc.tile_pool(name="aquart", bufs=2) as aqpool, \
             tc.tile_pool(name="oquart", bufs=2) as opool:
            for t in range(RT):
                nc8 = CB * KEEP
                creg = cand[:, t * nc8:(t + 1) * nc8]
                vv = vpool.tile([128, ((k + 7) // 8) * 8], F32, tag="vv",
                                name=f"vv{t}")
                work_a = mpool.tile([128, nc8], F32, tag="mwa", name=f"mwa{t}")
                work_b = mpool.tile([128, nc8], F32, tag="mwb", name=f"mwb{t}")
                src = creg
                rounds = (k + 7) // 8
                for r in range(rounds):
                    nc.vector.max(out=vv[:, r * 8:(r + 1) * 8], in_=src)
                    if r < rounds - 1:
                        dst = work_a if r % 2 == 0 else work_b
                        nc.vector.match_replace(
                            out=dst, in_to_replace=vv[:, r * 8:(r + 1) * 8],
                            in_values=src, imm_value=-1.0)
                        src = dst
                tau = vv[:, k - 1:k]
                for q in range(NQ):
                    q0 = q * QW
                    eq = epool.tile([128, QW], F32, tag="eq",
                                    name=f"eq{t}_{q}")
                    nc.sync.dma_start(
                        out=eq,
                        in_=esp_d[t * 128:(t + 1) * 128, q0:q0 + QW])
                    aq = aqpool.tile([128, QW], F32, tag="aq",
                                     name=f"aq{t}_{q}")
                    nc.scalar.activation(out=aq, in_=eq, func=AF.Abs)
                    oq = opool.tile([128, QW], F32, tag="oq",
                                    name=f"oq{t}_{q}")
                    # out = (|e| >= tau) * e
                    nc.vector.scalar_tensor_tensor(
                        out=oq, in0=aq, scalar=tau, in1=eq,
                        op0=OP.is_ge, op1=OP.mult)
                    nc.scalar.dma_start(
                        out=out_d[t * 128:(t + 1) * 128, q0:q0 + QW],
                        in_=oq)


def build_nc(rows, d_in, d_emb, k, has_bias=False):
    """Build the single-core Bass program (all 8 cores run it SPMD)."""
    assert rows % 128 == 0 and d_in % 256 == 0 and d_emb % STRIP == 0
    assert (d_emb // STRIP) * KEEP >= k

    nc = bacc.Bacc("TRN2", target_bir_lowering=False)

    # xt/w arrive pre-rounded to fp32r (12-bit mantissa) from the host;
    # declaring them float32r end-to-end satisfies the BIR verifier's
    # "matmul input must be rounded to FP32r" check.
    xt_d = nc.dram_tensor("xt", [d_in, rows], F32R, kind="ExternalInput")
    u8_d = nc.dram_tensor("u8", [d_in, rows], FP8, kind="ExternalInput")
    x_d = nc.dram_tensor("xn", [rows, d_in], F32, kind="ExternalInput")
    w_d = nc.dram_tensor("w", [d_in, d_emb], F32R, kind="ExternalInput")
    v8_d = nc.dram_tensor("v8", [d_in, d_emb], FP8, kind="ExternalInput")
    b_d = None
    if has_bias:
        # b pre-replicated to 128 partitions on the host (tiny)
        b_d = nc.dram_tensor("b", [128, d_emb], F32, kind="ExternalInput")
    out_d = nc.dram_tensor("out", [rows, d_emb], F32, kind="ExternalOutput")
    esp_d = nc.dram_tensor("espill", [rows, d_emb], F32)  # Internal scratch

    tensors = (xt_d, u8_d, x_d, w_d, v8_d, b_d, out_d, esp_d)
    with TileContext(nc) as tc:
        _emit(nc, tc, tensors, rows, d_in, d_emb, k, has_bias)
    nc.compile()  # bacc legalization: wait-splitting via event sems, etc.
    return nc


def round_fp32r(a):
    """Round fp32 -> fp32r (round-to-nearest to 11-bit mantissa), matching
    walrus's fp32_to_fp32r."""
    u = np.ascontiguousarray(a).view(np.uint32).astype(np.uint64)
    r = ((u + 0x800) & 0xFFFFF000) & 0xFFFFFFFF
    return r.astype(np.uint32).view(np.float32)


def to_f8(a):
    import ml_dtypes
    return a.astype(ml_dtypes.float8_e4m3fn)


_NC_CACHE = {}


def _get_nc(rows, d_in, d_emb, k, precise=False, has_bias=False):
    del precise  # single code path; kept for test.py compatibility
    key = (rows, d_in, d_emb, k, has_bias)
    if key not in _NC_CACHE:
        _NC_CACHE[key] = build_nc(rows, d_in, d_emb, k, has_bias=has_bias)
    return _NC_CACHE[key]


def kernel(x, W, b, k, _trace=False, _precise=None):
    """Full-input entry point: shards across 8 NeuronCores internally."""
    x = np.asarray(x, dtype=np.float32)
    W = np.ascontiguousarray(np.asarray(W, dtype=np.float32))
    b = np.ascontiguousarray(np.asarray(b, dtype=np.float32)).reshape(1, -1)
    kk = int(np.asarray(k))
    Bfull, d_in = x.shape
    d_emb = W.shape[1]
    assert (Bfull, d_in, d_emb, kk) == (B, D_IN, D_EMB, K), (
        f"kernel hardcoded for {(B, D_IN, D_EMB, K)}, got "
        f"{(Bfull, d_in, d_emb, kk)}")

    has_bias = bool(np.any(b))
    rows = Bfull // N_CORES
    nc = _get_nc(rows, d_in, d_emb, kk, has_bias=has_bias)

    Wr = round_fp32r(W)
    # V8 = c*(Wr + s*(W - Wr)) = c*((1-s)*Wr + s*W), computed blockwise
    import ml_dtypes
    V8 = np.empty((d_in, d_emb), dtype=ml_dtypes.float8_e4m3fn)
    for r0 in range(0, d_in, 512):
        blk = CC * ((1.0 - CS) * Wr[r0:r0 + 512] + CS * W[r0:r0 + 512])
        V8[r0:r0 + 512] = blk.astype(ml_dtypes.float8_e4m3fn)
    base = {"w": Wr, "v8": V8}
    if has_bias:
        base["b"] = np.ascontiguousarray(
            np.broadcast_to(b, (128, d_emb)))
    in_maps = []
    for c in range(N_CORES):
        xc = np.ascontiguousarray(x[c * rows:(c + 1) * rows])
        xct = np.ascontiguousarray(xc.T)
        xr = round_fp32r(xct)
        u8 = to_f8(xr + CS * (xct - xr))
        m = {"xt": xr, "u8": u8, "xn": xc, **base}
        in_maps.append(m)
    res = run_bass_kernel_spmd(
        nc, in_maps, core_ids=list(range(N_CORES)), trace=_trace)
    out = np.concatenate([res.results[c]["out"] for c in range(N_CORES)],
                         axis=0)
    if _trace:
        return out, res
    return out
